# Initial kernel scaffold
#
"""Trainium2 Bass kernel for nn_MetaLearner_25056839205556.

Meta-learner forward: for each param tensor P with grad G and meta-weights
w = [[w_g, w_p]]:
    out = P + (w_g * G + w_p * P) = (1 + w_p) * P + w_g * G

Pure elementwise over ~45.1M f32 elements (6 param/grad pairs).  Strategy:
flatten + concatenate all params (and grads) into one stream, pad to
8 * 16 * 128 * 2752 elements, and give each of the 8 NeuronCores an equal
contiguous chunk shaped (16, 128, 2752).  The tiny meta-weights are
replicated to all cores (broadcast to 128 partitions on the host — pure
data movement; all arithmetic, including alpha = 1 + w_p, runs on device).

Per core, per tile (128 x 2752 = 1.41 MB):
    load P tile, load G tile                (HWDGE DMA)
    P  = alpha * P                          (ACT engine, per-partition scale)
    P  = (G * beta) + P                     (DVE scalar_tensor_tensor)
    store P tile                            (HWDGE DMA)

HBM traffic is 3 x 22.5 MB per core => ~190 us at the ~358 GB/s
HBM-per-core limit; compute is ~32 us per engine, fully overlapped.
"""

import numpy as np

import concourse.bass as bass
import concourse.tile as tile
from concourse import bacc, mybir
from concourse import bass_utils

# Full (unsharded) input shapes, hardcoded from the problem spec.
_SHAPES = [(50257, 768), (768, 3072), (3072, 768), (768, 2304), (2304,), (768,)]
_SIZES = [int(np.prod(s)) for s in _SHAPES]
_TOTAL = sum(_SIZES)  # 45,088,512

N_CORES = 8
NT = 16       # tiles per core
P = 128       # SBUF partitions
F = 2752      # tile free dim; 16*128*2752 = 5,636,096 elems/core
_PER_CORE = NT * P * F
_PADDED = N_CORES * _PER_CORE  # 45,088,768 (pad = 256 elems)

_F32 = mybir.dt.float32

_cache = {}


def _build_nc():
    """Build + compile the Bass program once per process."""
    if "nc" in _cache:
        return _cache["nc"]

    nc = bacc.Bacc(
        "TRN2",
        target_bir_lowering=False,
        debug=False,
        num_devices=N_CORES,
    )
    w_d = nc.dram_tensor("w", [P, 2], _F32, kind="ExternalInput").ap()
    p_d = nc.dram_tensor("p", [NT, P, F], _F32, kind="ExternalInput").ap()
    g_d = nc.dram_tensor("g", [NT, P, F], _F32, kind="ExternalInput").ap()
    o_d = nc.dram_tensor("o", [NT, P, F], _F32, kind="ExternalOutput").ap()

    with tile.TileContext(nc) as tc:
        with (
            tc.tile_pool(name="const", bufs=1) as cpool,
            tc.tile_pool(name="pp", bufs=3) as ppool,
            tc.tile_pool(name="gp", bufs=3) as gpool,
        ):
            w_sb = cpool.tile([P, 2], _F32)
            nc.sync.dma_start(w_sb[:], w_d[:])
            alpha = cpool.tile([P, 1], _F32)
            # alpha = 1 + w_p  (per partition; w column 1)
            nc.vector.tensor_scalar_add(alpha[:], w_sb[:, 1:2], 1.0)

            for i in range(NT):
                p_t = ppool.tile([P, F], _F32)
                nc.sync.dma_start(p_t[:], p_d[i, :, :])
                g_t = gpool.tile([P, F], _F32)
                nc.sync.dma_start(g_t[:], g_d[i, :, :])
                # p_t = alpha * p_t  on the ACT engine
                nc.scalar.mul(p_t[:], p_t[:], alpha[:])
                # p_t = (g_t * beta) + p_t  on the DVE (beta = w column 0)
                nc.vector.scalar_tensor_tensor(
                    p_t[:],
                    g_t[:],
                    w_sb[:, 0:1],
                    p_t[:],
                    mybir.AluOpType.mult,
                    mybir.AluOpType.add,
                )
                nc.sync.dma_start(o_d[i, :, :], p_t[:])

    nc.compile()
    _cache["nc"] = nc
    return nc


def _run(weights, params, grads, **spmd_kwargs):
    """Shard, run on 8 cores, gather.  Returns (outputs_tuple, results)."""
    nc = _build_nc()

    w_rep = np.ascontiguousarray(
        np.broadcast_to(np.asarray(weights, np.float32).reshape(1, 2), (P, 2))
    )

    def pack(tensors):
        flat = np.concatenate(
            [np.asarray(t, np.float32).ravel() for t in tensors]
        )
        out = np.zeros(_PADDED, np.float32)
        out[:_TOTAL] = flat
        return out.reshape(N_CORES, NT, P, F)

    p_sh = pack(params)
    g_sh = pack(grads)

    in_maps = [
        {"w": w_rep, "p": p_sh[c], "g": g_sh[c]} for c in range(N_CORES)
    ]
    res = bass_utils.run_bass_kernel_spmd(
        nc, in_maps, core_ids=list(range(N_CORES)), **spmd_kwargs
    )

    full = np.concatenate([r["o"].ravel() for r in res.results])[:_TOTAL]
    outs = []
    off = 0
    for shape, size in zip(_SHAPES, _SIZES):
        outs.append(full[off : off + size].reshape(shape))
        off += size
    return tuple(outs), res


def kernel(weights, p0, p1, p2, p3, p4, p5, g0, g1, g2, g3, g4, g5):
    outs, _ = _run(weights, (p0, p1, p2, p3, p4, p5), (g0, g1, g2, g3, g4, g5))
    return outs


# revision 2
# speedup vs baseline: 2.2693x; 2.2693x over previous
"""Trainium2 Bass kernel for nn_MetaLearner_25056839205556.

Meta-learner forward: for each param tensor P with grad G and meta-weights
w = [[w_g, w_p]]:
    out = P + (w_g * G + w_p * P) = (1 + w_p) * P + w_g * G

Pure elementwise over ~45.1M f32 elements (6 param/grad pairs).  Strategy:
flatten + concatenate all params (and grads) into one stream, pad to
8 * 16 * 128 * 2752 elements, and give each of the 8 NeuronCores an equal
contiguous chunk shaped (16, 128, 2752).  The tiny meta-weights are
replicated to all cores (broadcast to 128 partitions on the host — pure
data movement; all arithmetic, including alpha = 1 + w_p, runs on device).

Per core, per tile (128 x 2752 = 1.41 MB):
    load P tile, load G tile                (HWDGE DMA)
    P  = alpha * P                          (ACT engine, per-partition scale)
    P  = (G * beta) + P                     (DVE scalar_tensor_tensor)
    store P tile                            (HWDGE DMA)

HBM traffic is 3 x 22.5 MB per core => ~190 us at the ~358 GB/s
HBM-per-core limit; compute is ~32 us per engine, fully overlapped.
"""

import numpy as np

import concourse.bass as bass
import concourse.tile as tile
from concourse import bacc, mybir
from concourse import bass_utils

# Full (unsharded) input shapes, hardcoded from the problem spec.
_SHAPES = [(50257, 768), (768, 3072), (3072, 768), (768, 2304), (2304,), (768,)]
_SIZES = [int(np.prod(s)) for s in _SHAPES]
_TOTAL = sum(_SIZES)  # 45,088,512

N_CORES = 8
NT = 16       # tiles per core
P = 128       # SBUF partitions
F = 2752      # tile free dim; 16*128*2752 = 5,636,096 elems/core
_PER_CORE = NT * P * F
_PADDED = N_CORES * _PER_CORE  # 45,088,768 (pad = 256 elems)

_F32 = mybir.dt.float32

_cache = {}


def _build_nc(repeats: int = 1):
    """Build + compile the Bass program once per process.

    ``repeats`` > 1 re-runs the whole tile loop that many times inside one
    NEFF — used only by the timing harness (wall-clock differencing); the
    result is identical since every pass writes the same outputs.
    """
    key = ("nc", repeats)
    if key in _cache:
        return _cache[key]

    nc = bacc.Bacc(
        "TRN2",
        target_bir_lowering=False,
        debug=False,
        num_devices=N_CORES,
    )
    w_d = nc.dram_tensor("w", [P, 2], _F32, kind="ExternalInput").ap()
    p_d = nc.dram_tensor("p", [NT, P, F], _F32, kind="ExternalInput").ap()
    g_d = nc.dram_tensor("g", [NT, P, F], _F32, kind="ExternalInput").ap()
    o_d = nc.dram_tensor("o", [NT, P, F], _F32, kind="ExternalOutput").ap()

    with tile.TileContext(nc) as tc:
        with (
            tc.tile_pool(name="const", bufs=1) as cpool,
            tc.tile_pool(name="pp", bufs=3) as ppool,
            tc.tile_pool(name="gp", bufs=3) as gpool,
        ):
            w_sb = cpool.tile([P, 2], _F32)
            nc.sync.dma_start(w_sb[:], w_d[:])
            alpha = cpool.tile([P, 1], _F32)
            # alpha = 1 + w_p  (per partition; w column 1)
            nc.vector.tensor_scalar_add(alpha[:], w_sb[:, 1:2], 1.0)

            for _ in range(repeats):
                for i in range(NT):
                    p_t = ppool.tile([P, F], _F32)
                    nc.sync.dma_start(p_t[:], p_d[i, :, :])
                    g_t = gpool.tile([P, F], _F32)
                    nc.sync.dma_start(g_t[:], g_d[i, :, :])
                    # p_t = alpha * p_t  on the ACT engine
                    nc.scalar.mul(p_t[:], p_t[:], alpha[:])
                    # p_t = (g_t * beta) + p_t  on the DVE (beta = w column 0)
                    nc.vector.scalar_tensor_tensor(
                        p_t[:],
                        g_t[:],
                        w_sb[:, 0:1],
                        p_t[:],
                        mybir.AluOpType.mult,
                        mybir.AluOpType.add,
                    )
                    nc.sync.dma_start(o_d[i, :, :], p_t[:])

    nc.compile()
    _cache[key] = nc
    return nc


def _run(weights, params, grads, **spmd_kwargs):
    """Shard, run on 8 cores, gather.  Returns (outputs_tuple, results)."""
    nc = _build_nc()

    w_rep = np.ascontiguousarray(
        np.broadcast_to(np.asarray(weights, np.float32).reshape(1, 2), (P, 2))
    )

    def pack(tensors):
        flat = np.concatenate(
            [np.asarray(t, np.float32).ravel() for t in tensors]
        )
        out = np.zeros(_PADDED, np.float32)
        out[:_TOTAL] = flat
        return out.reshape(N_CORES, NT, P, F)

    p_sh = pack(params)
    g_sh = pack(grads)

    in_maps = [
        {"w": w_rep, "p": p_sh[c], "g": g_sh[c]} for c in range(N_CORES)
    ]
    res = bass_utils.run_bass_kernel_spmd(
        nc, in_maps, core_ids=list(range(N_CORES)), **spmd_kwargs
    )

    full = np.concatenate([r["o"].ravel() for r in res.results])[:_TOTAL]
    outs = []
    off = 0
    for shape, size in zip(_SHAPES, _SIZES):
        outs.append(full[off : off + size].reshape(shape))
        off += size
    return tuple(outs), res


def kernel(weights, p0, p1, p2, p3, p4, p5, g0, g1, g2, g3, g4, g5):
    outs, _ = _run(weights, (p0, p1, p2, p3, p4, p5), (g0, g1, g2, g3, g4, g5))
    return outs
